# revision 1
# baseline (speedup 1.0000x reference)
"""CosineEmbeddingLoss (B=8192, D=128) on 8 TRN2 NeuronCores.

Data-parallel: each core gets a [1024,128] anchor slab + the full
[8192,128] positive matrix (bf16). Per core:
  - normalize positive rows (ttr sumsq -> rsqrt -> scale), DMA-xbar
    transpose to pT [128, 8192] bf16
  - transpose raw anchor slab to aT [128, 1024] bf16 (row scale folded
    in after the reduction: relu(c*x) = c*relu(x) for c>0)
  - 128 bf16 matmuls [K=128, M=128, N=512] -> PSUM [128,2048] groups
  - relu+row-sum of each group on ScalarE (activation Relu accum_out)
    or VectorE (tensor_tensor_reduce max/add), split for balance
  - diagonal correction from row-dots of matching anchor/positive rows
Host sums the 8 partial scalars, adds B (the +1 per diagonal term) and
divides by B*B.
"""

import numpy as np
import ml_dtypes

import concourse.bass as bass
import concourse.tile as tile
from concourse import bacc, mybir
from concourse.bass_utils import run_bass_kernel_spmd

B, D, NCORES = 8192, 128, 8
SLAB = B // NCORES          # 1024 anchor rows per core
PT = B // 128               # 64 positive tiles of 128 rows
AT = SLAB // 128            # 8 anchor tiles
NGRP = 4                    # [128, 2048] psum groups per m-block
GRPN = 2048
MMN = 512                   # matmul free dim
F32 = mybir.dt.float32
BF16 = mybir.dt.bfloat16

_CACHE: dict = {}


def _use_act(idx: int) -> bool:
    # ~18 of 32 groups on ScalarE (570ns/group) vs VectorE (658ns/group)
    return (idx * 9) // 16 != ((idx + 1) * 9) // 16


def _body(tc, a_in, p_in, pd_in, out):
    nc = tc.nc
    Relu = mybir.ActivationFunctionType.Relu
    Sqrt = mybir.ActivationFunctionType.Sqrt
    Square = mybir.ActivationFunctionType.Square
    mult, add, amax = mybir.AluOpType.mult, mybir.AluOpType.add, mybir.AluOpType.max
    sub = mybir.AluOpType.subtract
    X = mybir.AxisListType.X

    import contextlib
    ctx = contextlib.ExitStack()
    with ctx:
        singles = ctx.enter_context(tc.tile_pool(name="singles", bufs=1))
        ptiles = ctx.enter_context(tc.tile_pool(name="ptiles", bufs=6))
        phat = ctx.enter_context(tc.tile_pool(name="phat", bufs=6))
        junkp = ctx.enter_context(tc.tile_pool(name="junkp", bufs=3))
        prep_ctx = contextlib.ExitStack()
        tpsum = prep_ctx.enter_context(
            tc.tile_pool(name="tpsum", bufs=4, space="PSUM"))

        # persistent buffers
        pT = singles.tile([128, B], BF16)            # transposed normalized positive
        aT = singles.tile([128, SLAB], BF16)         # transposed raw anchor
        sumsq_p = singles.tile([128, PT], F32)
        rsq_p = singles.tile([128, PT], F32)
        sumsq_a = singles.tile([128, AT], F32)
        rsq_a = singles.tile([128, AT], F32)
        sumsq_pd = singles.tile([128, AT], F32)
        rsq_pd = singles.tile([128, AT], F32)
        draw = singles.tile([128, AT], F32)          # raw diag dots
        racc_a = singles.tile([128, 32], F32)        # ScalarE group sums
        racc_d = singles.tile([128, 32], F32)        # VectorE group sums
        zeros = singles.tile([128, GRPN], BF16)
        dummy = singles.tile([128, 1], F32)
        sqscr = singles.tile([128, D], BF16)
        sqf32 = singles.tile([128, D], F32)
        from concourse.masks import make_identity
        ident = singles.tile([128, 128], BF16)
        make_identity(nc, ident[:])
        nc.vector.memset(racc_a[:], 0.0)
        nc.vector.memset(racc_d[:], 0.0)
        nc.vector.memset(zeros[:], 0.0)

        p_r = p_in.rearrange("(n p) d -> n p d", p=128)
        a_r = a_in.rearrange("(n p) d -> n p d", p=128)
        pd_r = pd_in.rearrange("(n p) d -> n p d", p=128)

        # ---- positive: load+sumsq per 16-tile batch, rsqrt, scale+transpose ----
        p_nat = singles.tile([128, B], BF16)
        for q in range(PT // 16):
            for t in range(q * 16, (q + 1) * 16):
                pn = p_nat[:, t * 128 : (t + 1) * 128]
                nc.sync.dma_start(out=pn, in_=p_r[t])
                nc.scalar.activation(
                    out=sqscr[:], in_=pn, func=Square,
                    accum_out=sumsq_p[:, t : t + 1])
            sl = slice(q * 16, (q + 1) * 16)
            nc.scalar.activation(
                out=rsq_p[:, sl], in_=sumsq_p[:, sl], func=Sqrt)
            nc.vector.reciprocal(out=rsq_p[:, sl], in_=rsq_p[:, sl])
            for t in range(q * 16, (q + 1) * 16):
                ph = phat.tile([128, D], BF16, tag="ph")
                nc.vector.tensor_scalar(
                    out=ph[:], in0=p_nat[:, t * 128 : (t + 1) * 128],
                    scalar1=rsq_p[:, t : t + 1], scalar2=None, op0=mult)
                tp = tpsum.tile([128, 128], BF16, tag="tp")
                nc.tensor.transpose(tp[:], ph[:], ident[:])
                nc.vector.tensor_copy(
                    out=pT[:, t * 128 : (t + 1) * 128], in_=tp[:])

        # ---- anchor: load, sumsq, transpose raw ----
        for t in range(AT):
            at = ptiles.tile([128, D], BF16, tag="at")
            nc.sync.dma_start(out=at[:], in_=a_r[t])
            nc.scalar.activation(
                out=sqscr[:], in_=at[:], func=Square,
                accum_out=sumsq_a[:, t : t + 1])
            tp = tpsum.tile([128, 128], BF16, tag="tp")
            nc.tensor.transpose(tp[:], at[:], ident[:])
            nc.vector.tensor_copy(
                out=aT[:, t * 128 : (t + 1) * 128], in_=tp[:])
            # matching positive rows for the diagonal
            pdt = ptiles.tile([128, D], BF16, tag="pdt")
            nc.sync.dma_start(out=pdt[:], in_=pd_r[t])
            nc.scalar.activation(
                out=sqscr[:], in_=pdt[:], func=Square,
                accum_out=sumsq_pd[:, t : t + 1])
            nc.vector.tensor_tensor(out=sqf32[:], in0=at[:], in1=pdt[:], op=mult)
            nc.vector.tensor_reduce(
                out=draw[:, t : t + 1], in_=sqf32[:], axis=X, op=add)
        nc.scalar.activation(out=rsq_a[:], in_=sumsq_a[:], func=Sqrt)
        nc.vector.reciprocal(out=rsq_a[:], in_=rsq_a[:])
        nc.scalar.activation(out=rsq_pd[:], in_=sumsq_pd[:], func=Sqrt)
        nc.vector.reciprocal(out=rsq_pd[:], in_=rsq_pd[:])

        prep_ctx.close()
        psum = ctx.enter_context(tc.tile_pool(name="psum", bufs=2, space="PSUM"))

        # ---- main loop ----
        for g in range(NGRP):
            for m in range(AT):
                ps = psum.tile([128, GRPN], F32, tag="ps")
                for j in range(GRPN // MMN):
                    col = g * GRPN + j * MMN
                    nc.tensor.matmul(
                        out=ps[:, j * MMN : (j + 1) * MMN],
                        lhsT=aT[:, m * 128 : (m + 1) * 128],
                        rhs=pT[:, col : col + MMN],
                        start=True, stop=True)
                idx = g * AT + m
                junk = junkp.tile([128, GRPN], BF16, tag="junk")
                if idx % 3 != 0:
                    nc.scalar.activation(
                        out=junk[:], in_=ps[:], func=Relu,
                        accum_out=racc_a[:, idx : idx + 1])
                else:
                    nc.vector.tensor_scalar(
                        out=junk[:], in0=ps[:], scalar1=0.0, scalar2=None,
                        op0=amax)
                    nc.vector.tensor_reduce(
                        out=racc_d[:, idx : idx + 1], in_=junk[:], axis=X,
                        op=add)

        # ---- combine ----
        racc_s = singles.tile([128, 32], F32)
        nc.vector.tensor_add(racc_s[:], racc_a[:], racc_d[:])
        rowsum = singles.tile([128, AT], F32)
        racc3 = racc_s.rearrange("p (g m) -> p g m", g=NGRP)
        nc.vector.tensor_add(rowsum[:], racc3[:, 0, :], racc3[:, 1, :])
        nc.vector.tensor_add(rowsum[:], rowsum[:], racc3[:, 2, :])
        nc.vector.tensor_add(rowsum[:], rowsum[:], racc3[:, 3, :])
        # scale relu-sums by r_a; diag cos = draw * r_a * r_pd
        nc.vector.tensor_mul(rowsum[:], rowsum[:], rsq_a[:])
        dcos = singles.tile([128, AT], F32)
        nc.vector.tensor_mul(dcos[:], draw[:], rsq_a[:])
        nc.vector.tensor_mul(dcos[:], dcos[:], rsq_pd[:])
        drelu = singles.tile([128, AT], F32)
        nc.scalar.activation(out=drelu[:], in_=dcos[:], func=Relu)
        # contrib = rowsum - dcos - drelu   (the +1 per diag added on host)
        nc.vector.tensor_tensor(rowsum[:], rowsum[:], dcos[:], op=sub)
        nc.vector.tensor_tensor(rowsum[:], rowsum[:], drelu[:], op=sub)
        total = singles.tile([128, 1], F32)
        nc.vector.tensor_reduce(total[:], rowsum[:], axis=X, op=add)
        from concourse.bass_isa import ReduceOp
        nc.gpsimd.partition_all_reduce(total[:], total[:], 128, ReduceOp.add)
        nc.sync.dma_start(out=out[:], in_=total[0:1, 0:1])


def _build():
    nc = bacc.Bacc("TRN2", target_bir_lowering=False, debug=False,
                   num_devices=NCORES)
    a_in = nc.declare_dram_parameter("a", [SLAB, D], BF16, isOutput=False)
    p_in = nc.declare_dram_parameter("p", [B, D], BF16, isOutput=False)
    pd_in = nc.declare_dram_parameter("pd", [SLAB, D], BF16, isOutput=False)
    out = nc.declare_dram_parameter("out", [1, 1], F32, isOutput=True)
    with tile.TileContext(nc) as tc:
        _body(tc, a_in[:], p_in[:], pd_in[:], out[:])
    nc.compile()
    return nc


def kernel(hid_positive: np.ndarray, hid_anchor: np.ndarray, **run_kwargs):
    if "nc" not in _CACHE:
        _CACHE["nc"] = _build()
    nc = _CACHE["nc"]
    p16 = np.asarray(hid_positive, dtype=np.float32).astype(ml_dtypes.bfloat16)
    a16 = np.asarray(hid_anchor, dtype=np.float32).astype(ml_dtypes.bfloat16)
    in_maps = []
    for c in range(NCORES):
        sl = slice(c * SLAB, (c + 1) * SLAB)
        in_maps.append({"a": a16[sl], "p": p16, "pd": p16[sl]})
    res = run_bass_kernel_spmd(nc, in_maps, core_ids=list(range(NCORES)),
                               **run_kwargs)
    s = sum(float(res.results[c]["out"][0, 0]) for c in range(NCORES))
    loss = np.float32((s + B) / (float(B) * float(B)))
    if run_kwargs:
        _CACHE["last_result"] = res
    return np.asarray(loss, dtype=np.float32)



# revision 2
# speedup vs baseline: 1.0519x; 1.0519x over previous
"""CosineEmbeddingLoss (B=8192, D=128) on 8 TRN2 NeuronCores.

Flipped data-parallel layout: each core takes a [1024,128] anchor slab
and the FULL positive matrix, transposed on load via DMA-XBAR (raw,
un-normalized).  Blocks are [128 positives x 1024 anchors]:

  raw[j, i] = p_j . (a_i/|a_i|)        (64 matmuls, lhsT = pT block)

Since relu(c*x) = c*relu(x) for c > 0, the positive-norm scale 1/|p_j|
is a per-partition scalar of each block's row-sum and is applied on the
HOST after the fused relu+row-sum accumulation:

  racc[p, t] = sum_i relu(raw[128t+p, i])     (one instr per block,
               split across ScalarE activation(Relu, accum_out) and
               VectorE tensor_scalar(max 0, accum add))

Each core also computes sum(p_j^2) and dhat_i = (a_i/|a_i|) . p_i for
its own 1024-row slab (DVE scalar_tensor_tensor with accum).  Host
assembles the full 1/|p| vector, scales+sums racc, removes the diagonal
relu terms and adds the (1 - cos_ii) diagonal terms.
"""

import numpy as np
import ml_dtypes

import concourse.bass as bass
import concourse.tile as tile
from concourse import bacc, mybir
from concourse.bass_utils import run_bass_kernel_spmd

B, D, NCORES = 8192, 128, 8
SLAB = B // NCORES          # 1024 anchors per core
PT = B // 128               # 64 positive blocks
AT = SLAB // 128            # 8 anchor tiles
MMN = 512                   # matmul free-dim chunk
XCH = 32                    # pT xbar-transpose DMA chunks
F32 = mybir.dt.float32
BF16 = mybir.dt.bfloat16

_CACHE: dict = {}


def _dve_blocks():
    # DVE takes ~40% of main blocks (rate 1360ns vs Act 1005ns per
    # [128,1024] f32 psum block, DVE also carries ~6us of prep).
    # First few blocks go to Act while DVE drains its prep queue.
    share = 0.40
    sel = set()
    acc = 0.0
    for t in range(PT):
        if t < 5:
            continue
        acc += share
        if acc >= 1.0:
            acc -= 1.0
            sel.add(t)
    return sel


DVE_BLOCKS = _dve_blocks()


def _body(tc, a_in, p_in, ps_in, racc_d_o, racc_a_o, ssq_p_o, dhat_o):
    nc = tc.nc
    Relu = mybir.ActivationFunctionType.Relu
    Sqrt = mybir.ActivationFunctionType.Sqrt
    mult = mybir.AluOpType.mult
    add = mybir.AluOpType.add
    amax = mybir.AluOpType.max
    byp = mybir.AluOpType.bypass

    import contextlib
    ctx = contextlib.ExitStack()
    with ctx:
        singles = ctx.enter_context(tc.tile_pool(name="singles", bufs=1))
        junkd = ctx.enter_context(tc.tile_pool(name="junkd", bufs=2))
        junka = ctx.enter_context(tc.tile_pool(name="junka", bufs=2))
        psum = ctx.enter_context(tc.tile_pool(name="psum", bufs=4, space="PSUM"))

        pT = singles.tile([128, B], BF16)         # full positives, transposed
        aT = singles.tile([128, SLAB], BF16)      # normalized anchors, transposed
        a_all = singles.tile([128, SLAB], BF16)   # raw anchor tiles (row-major)
        ah_all = singles.tile([128, SLAB], BF16)  # normalized anchors (row-major)
        ps_all = singles.tile([128, SLAB], BF16)  # own positive slab (row-major)
        ssq_a = singles.tile([128, AT], F32)
        rsq_a = singles.tile([128, AT], F32)
        ssq_p = singles.tile([128, AT], F32)
        dhat = singles.tile([128, AT], F32)
        racc_d = singles.tile([128, PT], F32)
        racc_a = singles.tile([128, PT], F32)
        junk_s = singles.tile([128, 128], BF16)

        a_r = a_in.rearrange("(n p) d -> n p d", p=128)
        ps_r = ps_in.rearrange("(n p) d -> n p d", p=128)

        # ---- input DMAs ----
        for t in range(AT):
            nc.sync.dma_start(out=a_all[:, t * 128:(t + 1) * 128], in_=a_r[t])
        for t in range(AT):
            nc.sync.dma_start(out=ps_all[:, t * 128:(t + 1) * 128], in_=ps_r[t])
        xw = B // XCH  # 256 positive rows per xbar chunk
        for k in range(XCH):
            nc.sync.dma_start_transpose(
                pT[:, k * xw:(k + 1) * xw], p_in[k * xw:(k + 1) * xw, :])

        # ---- anchor normalization ----
        for t in range(AT):
            at = a_all[:, t * 128:(t + 1) * 128]
            nc.vector.scalar_tensor_tensor(
                out=junk_s[:], in0=at, scalar=1.0, in1=at,
                op0=byp, op1=mult, accum_out=ssq_a[:, t:t + 1])
        nc.scalar.activation(out=rsq_a[:], in_=ssq_a[:], func=Sqrt)
        nc.vector.reciprocal(out=rsq_a[:], in_=rsq_a[:])
        for t in range(AT):
            nc.vector.tensor_scalar(
                out=ah_all[:, t * 128:(t + 1) * 128],
                in0=a_all[:, t * 128:(t + 1) * 128],
                scalar1=rsq_a[:, t:t + 1], scalar2=None, op0=mult)
        for t in range(AT):
            nc.sync.dma_start_transpose(
                aT[:, t * 128:(t + 1) * 128],
                ah_all[:, t * 128:(t + 1) * 128])

        # ---- own-slab positive sumsq + diag dots (DVE, fills pipeline gaps) ----
        for t in range(AT):
            pst = ps_all[:, t * 128:(t + 1) * 128]
            nc.vector.scalar_tensor_tensor(
                out=junk_s[:], in0=pst, scalar=1.0, in1=pst,
                op0=byp, op1=mult, accum_out=ssq_p[:, t:t + 1])
            nc.vector.scalar_tensor_tensor(
                out=junk_s[:], in0=ah_all[:, t * 128:(t + 1) * 128],
                scalar=1.0, in1=pst,
                op0=byp, op1=mult, accum_out=dhat[:, t:t + 1])

        # ---- main loop: 64 blocks of [128 positives, 1024 anchors] ----
        for t in range(PT):
            ps = psum.tile([128, SLAB], F32, tag="mm")
            lhsT = pT[:, t * 128:(t + 1) * 128]
            for j in range(SLAB // MMN):
                nc.tensor.matmul(
                    out=ps[:, j * MMN:(j + 1) * MMN],
                    lhsT=lhsT, rhs=aT[:, j * MMN:(j + 1) * MMN],
                    start=True, stop=True)
            if t in DVE_BLOCKS:
                junk = junkd.tile([128, SLAB], BF16, tag="jd")
                nc.vector.tensor_scalar(
                    out=junk[:], in0=ps[:], scalar1=0.0, scalar2=None,
                    op0=amax, op1=add, accum_out=racc_d[:, t:t + 1])
            else:
                junk = junka.tile([128, SLAB], BF16, tag="ja")
                nc.scalar.activation(
                    out=junk[:], in_=ps[:], func=Relu,
                    accum_out=racc_a[:, t:t + 1])

        # ---- outputs ----
        nc.sync.dma_start(out=racc_d_o[:], in_=racc_d[:])
        nc.sync.dma_start(out=racc_a_o[:], in_=racc_a[:])
        nc.sync.dma_start(out=ssq_p_o[:], in_=ssq_p[:])
        nc.sync.dma_start(out=dhat_o[:], in_=dhat[:])


def _build():
    nc = bacc.Bacc("TRN2", target_bir_lowering=False, debug=False,
                   num_devices=NCORES)
    a_in = nc.declare_dram_parameter("a", [SLAB, D], BF16, isOutput=False)
    p_in = nc.declare_dram_parameter("p", [B, D], BF16, isOutput=False)
    ps_in = nc.declare_dram_parameter("ps", [SLAB, D], BF16, isOutput=False)
    racc_d_o = nc.declare_dram_parameter("racc_d", [128, PT], F32, isOutput=True)
    racc_a_o = nc.declare_dram_parameter("racc_a", [128, PT], F32, isOutput=True)
    ssq_p_o = nc.declare_dram_parameter("ssq_p", [128, AT], F32, isOutput=True)
    dhat_o = nc.declare_dram_parameter("dhat", [128, AT], F32, isOutput=True)
    with tile.TileContext(nc) as tc:
        _body(tc, a_in[:], p_in[:], ps_in[:], racc_d_o[:], racc_a_o[:],
              ssq_p_o[:], dhat_o[:])
    nc.compile()
    return nc


def kernel(hid_positive: np.ndarray, hid_anchor: np.ndarray, **run_kwargs):
    if "nc" not in _CACHE:
        _CACHE["nc"] = _build()
    nc = _CACHE["nc"]
    p16 = np.asarray(hid_positive, dtype=np.float32).astype(ml_dtypes.bfloat16)
    a16 = np.asarray(hid_anchor, dtype=np.float32).astype(ml_dtypes.bfloat16)
    in_maps = []
    for c in range(NCORES):
        sl = slice(c * SLAB, (c + 1) * SLAB)
        in_maps.append({"a": a16[sl], "p": p16, "ps": p16[sl]})
    res = run_bass_kernel_spmd(nc, in_maps, core_ids=list(range(NCORES)),
                               **run_kwargs)
    # host: assemble 1/|p_j| from per-core slab sumsq
    ssq_full = np.empty(B, dtype=np.float64)
    for c in range(NCORES):
        arr = np.asarray(res.results[c]["ssq_p"], dtype=np.float64)  # [128, 8]
        ssq_full[c * SLAB:(c + 1) * SLAB] = arr.T.reshape(SLAB)
    rsq = 1.0 / np.maximum(np.sqrt(ssq_full), 1e-8)
    rsq_mat = rsq.reshape(PT, 128).T  # [128, 64]; [p, t] -> row 128t+p

    total = 0.0
    diag_relu = 0.0
    diag_cos = 0.0
    for c in range(NCORES):
        rd = np.asarray(res.results[c]["racc_d"], dtype=np.float64)
        ra = np.asarray(res.results[c]["racc_a"], dtype=np.float64)
        racc = ra
        for t in DVE_BLOCKS:
            racc[:, t] = rd[:, t]
        total += float((racc * rsq_mat).sum())
        dh = np.asarray(res.results[c]["dhat"], dtype=np.float64)  # [128, 8]
        # dhat[p, m] -> anchor/positive index 1024c + 128m + p
        r_slab = rsq[c * SLAB:(c + 1) * SLAB].reshape(AT, 128).T  # [128, 8]
        dcos = dh * r_slab
        diag_relu += float(np.maximum(dcos, 0.0).sum())
        diag_cos += float(dcos.sum())
    loss = (total - diag_relu - diag_cos + B) / (float(B) * float(B))
    if run_kwargs:
        _CACHE["last_result"] = res
    return np.asarray(loss, dtype=np.float32)


# revision 4
# speedup vs baseline: 1.8267x; 1.7366x over previous
"""CosineEmbeddingLoss (B=8192, D=128) on 8 TRN2 NeuronCores.

Flipped data-parallel layout: each core takes a [1024,128] anchor slab
and the FULL positive matrix, transposed on load via DMA-XBAR (raw,
un-normalized).  Blocks are [128 positives x 1024 anchors]:

  raw[j, i] = p_j . (a_i/|a_i|)        (64 matmuls, lhsT = pT block)

Since relu(c*x) = c*relu(x) for c > 0, the positive-norm scale 1/|p_j|
is a per-partition scalar of each block's row-sum and is applied on the
HOST after the fused relu+row-sum accumulation:

  racc[p, t] = sum_i relu(raw[128t+p, i])     (one instr per block,
               split across ScalarE activation(Relu, accum_out) and
               VectorE tensor_scalar(max 0, accum add))

Each core also computes sum(p_j^2) and dhat_i = (a_i/|a_i|) . p_i for
its own 1024-row slab (DVE scalar_tensor_tensor with accum).  Host
assembles the full 1/|p| vector, scales+sums racc, removes the diagonal
relu terms and adds the (1 - cos_ii) diagonal terms.
"""

import numpy as np
import ml_dtypes

import concourse.bass as bass
import concourse.tile as tile
from concourse import bacc, mybir
from concourse.bass_utils import run_bass_kernel_spmd

B, D, NCORES = 8192, 128, 8
SLAB = B // NCORES          # 1024 anchors per core
PT = B // 128               # 64 positive blocks
AT = SLAB // 128            # 8 anchor tiles
MMN = 512                   # matmul free-dim chunk
XCH = 16                    # pT xbar-transpose DMA chunks
F32 = mybir.dt.float32
BF16 = mybir.dt.bfloat16

_CACHE: dict = {}


def _dve_blocks():
    # DVE takes ~47% of main blocks (measured 1232ns vs Act 1208ns per
    # [128,1024] f32 psum block; DVE also carries ~3us of in-window prep).
    share = 0.47
    sel = set()
    acc = 0.0
    for t in range(PT):
        if t < 3:
            continue
        acc += share
        if acc >= 1.0:
            acc -= 1.0
            sel.add(t)
    return sel


DVE_BLOCKS = _dve_blocks()


def _body(tc, a_in, p_in, ps_in, racc_d_o, racc_a_o, ssq_p_o, dhat_o):
    nc = tc.nc
    Relu = mybir.ActivationFunctionType.Relu
    Sqrt = mybir.ActivationFunctionType.Sqrt
    mult = mybir.AluOpType.mult
    add = mybir.AluOpType.add
    amax = mybir.AluOpType.max
    byp = mybir.AluOpType.bypass

    import contextlib
    ctx = contextlib.ExitStack()
    with ctx:
        singles = ctx.enter_context(tc.tile_pool(name="singles", bufs=1))
        junkd = ctx.enter_context(tc.tile_pool(name="junkd", bufs=2))
        junka = ctx.enter_context(tc.tile_pool(name="junka", bufs=2))
        prep_ctx = contextlib.ExitStack()
        tpsum = prep_ctx.enter_context(
            tc.tile_pool(name="tpsum", bufs=2, space="PSUM"))

        pT = singles.tile([128, B], BF16)         # full positives, transposed
        aT = singles.tile([128, SLAB], BF16)      # normalized anchors, transposed
        a_all = singles.tile([128, SLAB], BF16)   # raw anchor tiles (row-major)
        ah_all = singles.tile([128, SLAB], BF16)  # normalized anchors (row-major)
        ps_all = singles.tile([128, SLAB], BF16)  # own positive slab (row-major)
        ssq_a = singles.tile([128, AT], F32)
        rsq_a = singles.tile([128, AT], F32)
        ssq_p = singles.tile([128, AT], F32)
        dhat = singles.tile([128, AT], F32)
        racc_d = singles.tile([128, PT], F32)
        racc_a = singles.tile([128, PT], F32)
        junk_s = singles.tile([128, 128], BF16)
        from concourse.masks import make_identity
        ident = singles.tile([128, 128], BF16)
        make_identity(nc, ident[:])

        # per-partition-major DRAM views so multi-tile DMAs enumerate in
        # the same order as the SBUF destination [p, t, d]
        a_pm = a_in.rearrange("(n p) d -> p n d", p=128)
        ps_pm = ps_in.rearrange("(n p) d -> p n d", p=128)
        a3 = a_all.rearrange("p (n d) -> p n d", d=128)
        ps3 = ps_all.rearrange("p (n d) -> p n d", d=128)

        # ---- input DMAs (SP issue is ~0.6us plain / ~1.25us xbar each;
        # order: anchors (critical prep chain) -> first pT chunks ->
        # positive slab -> remaining pT chunks) ----
        for k in range(4):
            nc.sync.dma_start(out=a3[:, 2 * k:2 * k + 2, :],
                              in_=a_pm[:, 2 * k:2 * k + 2, :])
        xw = B // XCH  # positive rows per xbar chunk
        for k in range(4):
            nc.sync.dma_start_transpose(
                pT[:, k * xw:(k + 1) * xw], p_in[k * xw:(k + 1) * xw, :])
        for k in range(2):
            nc.sync.dma_start(out=ps3[:, 4 * k:4 * k + 4, :],
                              in_=ps_pm[:, 4 * k:4 * k + 4, :])
        for k in range(4, XCH):
            nc.sync.dma_start_transpose(
                pT[:, k * xw:(k + 1) * xw], p_in[k * xw:(k + 1) * xw, :])

        # ---- anchor normalization ----
        for t in range(AT):
            at = a_all[:, t * 128:(t + 1) * 128]
            nc.vector.scalar_tensor_tensor(
                out=junk_s[:], in0=at, scalar=1.0, in1=at,
                op0=byp, op1=mult, accum_out=ssq_a[:, t:t + 1])
        nc.scalar.activation(out=rsq_a[:], in_=ssq_a[:], func=Sqrt)
        nc.vector.reciprocal(out=rsq_a[:], in_=rsq_a[:])
        for t in range(AT):
            nc.vector.tensor_scalar(
                out=ah_all[:, t * 128:(t + 1) * 128],
                in0=a_all[:, t * 128:(t + 1) * 128],
                scalar1=rsq_a[:, t:t + 1], scalar2=None, op0=mult)
        # PE transposes for the anchor tiles (PE is idle during prep);
        # psum->SBUF copies on Act, which is also idle pre-main-loop
        for t in range(AT):
            tp = tpsum.tile([128, 128], BF16, tag="tp")
            nc.tensor.transpose(tp[:], ah_all[:, t * 128:(t + 1) * 128],
                                ident[:])
            nc.scalar.copy(out=aT[:, t * 128:(t + 1) * 128], in_=tp[:])
        prep_ctx.close()
        psum = ctx.enter_context(tc.tile_pool(name="psum", bufs=4, space="PSUM"))

        def _prep_stt(t):
            pst = ps_all[:, t * 128:(t + 1) * 128]
            nc.vector.scalar_tensor_tensor(
                out=junk_s[:], in0=pst, scalar=1.0, in1=pst,
                op0=byp, op1=mult, accum_out=ssq_p[:, t:t + 1])
            nc.vector.scalar_tensor_tensor(
                out=junk_s[:], in0=ah_all[:, t * 128:(t + 1) * 128],
                scalar=1.0, in1=pst,
                op0=byp, op1=mult, accum_out=dhat[:, t:t + 1])

        # ---- main loop: 64 blocks of [128 positives, 1024 anchors];
        # own-slab sumsq/diag stt ops are woven into the DVE stream after
        # its first couple of EW blocks ----
        stt_after = {6: [0, 1], 8: [2, 3], 10: [4, 5], 12: [6, 7]}
        for t in range(PT):
            ps = psum.tile([128, SLAB], F32, tag="mm")
            lhsT = pT[:, t * 128:(t + 1) * 128]
            for j in range(SLAB // MMN):
                nc.tensor.matmul(
                    out=ps[:, j * MMN:(j + 1) * MMN],
                    lhsT=lhsT, rhs=aT[:, j * MMN:(j + 1) * MMN],
                    start=True, stop=True)
            if t in DVE_BLOCKS:
                junk = junkd.tile([128, SLAB], BF16, tag="jd")
                nc.vector.tensor_scalar(
                    out=junk[:], in0=ps[:], scalar1=0.0, scalar2=None,
                    op0=amax, op1=add, accum_out=racc_d[:, t:t + 1])
            else:
                junk = junka.tile([128, SLAB], BF16, tag="ja")
                nc.scalar.activation(
                    out=junk[:], in_=ps[:], func=Relu,
                    accum_out=racc_a[:, t:t + 1])
            for pt in stt_after.get(t, []):
                _prep_stt(pt)

        # ---- outputs ----
        nc.sync.dma_start(out=racc_d_o[:], in_=racc_d[:])
        nc.sync.dma_start(out=racc_a_o[:], in_=racc_a[:])
        nc.sync.dma_start(out=ssq_p_o[:], in_=ssq_p[:])
        nc.sync.dma_start(out=dhat_o[:], in_=dhat[:])


def _build():
    nc = bacc.Bacc("TRN2", target_bir_lowering=False, debug=False,
                   num_devices=NCORES)
    a_in = nc.declare_dram_parameter("a", [SLAB, D], BF16, isOutput=False)
    p_in = nc.declare_dram_parameter("p", [B, D], BF16, isOutput=False)
    ps_in = nc.declare_dram_parameter("ps", [SLAB, D], BF16, isOutput=False)
    racc_d_o = nc.declare_dram_parameter("racc_d", [128, PT], F32, isOutput=True)
    racc_a_o = nc.declare_dram_parameter("racc_a", [128, PT], F32, isOutput=True)
    ssq_p_o = nc.declare_dram_parameter("ssq_p", [128, AT], F32, isOutput=True)
    dhat_o = nc.declare_dram_parameter("dhat", [128, AT], F32, isOutput=True)
    with tile.TileContext(nc) as tc:
        _body(tc, a_in[:], p_in[:], ps_in[:], racc_d_o[:], racc_a_o[:],
              ssq_p_o[:], dhat_o[:])
    nc.compile()
    return nc


def kernel(hid_positive: np.ndarray, hid_anchor: np.ndarray, **run_kwargs):
    if "nc" not in _CACHE:
        _CACHE["nc"] = _build()
    nc = _CACHE["nc"]
    p16 = np.asarray(hid_positive, dtype=np.float32).astype(ml_dtypes.bfloat16)
    a16 = np.asarray(hid_anchor, dtype=np.float32).astype(ml_dtypes.bfloat16)
    in_maps = []
    for c in range(NCORES):
        sl = slice(c * SLAB, (c + 1) * SLAB)
        in_maps.append({"a": a16[sl], "p": p16, "ps": p16[sl]})
    res = run_bass_kernel_spmd(nc, in_maps, core_ids=list(range(NCORES)),
                               **run_kwargs)
    # host: assemble 1/|p_j| from per-core slab sumsq
    ssq_full = np.empty(B, dtype=np.float64)
    for c in range(NCORES):
        arr = np.asarray(res.results[c]["ssq_p"], dtype=np.float64)  # [128, 8]
        ssq_full[c * SLAB:(c + 1) * SLAB] = arr.T.reshape(SLAB)
    rsq = 1.0 / np.maximum(np.sqrt(ssq_full), 1e-8)
    rsq_mat = rsq.reshape(PT, 128).T  # [128, 64]; [p, t] -> row 128t+p

    total = 0.0
    diag_relu = 0.0
    diag_cos = 0.0
    for c in range(NCORES):
        rd = np.asarray(res.results[c]["racc_d"], dtype=np.float64)
        ra = np.asarray(res.results[c]["racc_a"], dtype=np.float64)
        racc = ra
        for t in DVE_BLOCKS:
            racc[:, t] = rd[:, t]
        total += float((racc * rsq_mat).sum())
        dh = np.asarray(res.results[c]["dhat"], dtype=np.float64)  # [128, 8]
        # dhat[p, m] -> anchor/positive index 1024c + 128m + p
        r_slab = rsq[c * SLAB:(c + 1) * SLAB].reshape(AT, 128).T  # [128, 8]
        dcos = dh * r_slab
        diag_relu += float(np.maximum(dcos, 0.0).sum())
        diag_cos += float(dcos.sum())
    loss = (total - diag_relu - diag_cos + B) / (float(B) * float(B))
    if run_kwargs:
        _CACHE["last_result"] = res
    return np.asarray(loss, dtype=np.float32)


# revision 5
# speedup vs baseline: 2.0219x; 1.1068x over previous
"""CosineEmbeddingLoss (B=8192, D=128) on 8 TRN2 NeuronCores.

Flipped data-parallel layout: each core takes a [1024,128] anchor slab
and the FULL positive matrix, transposed on load via DMA-XBAR (raw,
un-normalized).  Blocks are [128 positives x 1024 anchors]:

  raw[j, i] = p_j . (a_i/|a_i|)        (64 matmuls, lhsT = pT block)

Since relu(c*x) = c*relu(x) for c > 0, the positive-norm scale 1/|p_j|
is a per-partition scalar of each block's row-sum and is applied on the
HOST after the fused relu+row-sum accumulation:

  racc[p, t] = sum_i relu(raw[128t+p, i])     (one instr per block,
               split across ScalarE activation(Relu, accum_out) and
               VectorE tensor_scalar(max 0, accum add))

Each core also computes sum(p_j^2) and dhat_i = (a_i/|a_i|) . p_i for
its own 1024-row slab (DVE scalar_tensor_tensor with accum).  Host
assembles the full 1/|p| vector, scales+sums racc, removes the diagonal
relu terms and adds the (1 - cos_ii) diagonal terms.
"""

import numpy as np
import ml_dtypes

import concourse.bass as bass
import concourse.tile as tile
from concourse import bacc, mybir
from concourse.bass_utils import run_bass_kernel_spmd

B, D, NCORES = 8192, 128, 8
SLAB = B // NCORES          # 1024 anchors per core
PT = B // 128               # 64 positive blocks
AT = SLAB // 128            # 8 anchor tiles
MMN = 512                   # matmul free-dim chunk
XCH = 16                    # pT xbar-transpose DMA chunks
F32 = mybir.dt.float32
BF16 = mybir.dt.bfloat16

_CACHE: dict = {}


def _dve_blocks():
    # DVE takes ~47% of main blocks (measured 1232ns vs Act 1208ns per
    # [128,1024] f32 psum block; DVE also carries ~3us of in-window prep).
    share = 0.47
    sel = set()
    acc = 0.0
    for t in range(PT):
        if t < 3:
            continue
        acc += share
        if acc >= 1.0:
            acc -= 1.0
            sel.add(t)
    return sel


DVE_BLOCKS = _dve_blocks()


def _body(tc, a_in, p_in, ps_in, racc_d_o, racc_a_o, ssq_p_o, dhat_o):
    nc = tc.nc
    Relu = mybir.ActivationFunctionType.Relu
    Sqrt = mybir.ActivationFunctionType.Sqrt
    mult = mybir.AluOpType.mult
    add = mybir.AluOpType.add
    amax = mybir.AluOpType.max
    byp = mybir.AluOpType.bypass

    import contextlib
    ctx = contextlib.ExitStack()
    with ctx:
        singles = ctx.enter_context(tc.tile_pool(name="singles", bufs=1))
        junkd = ctx.enter_context(tc.tile_pool(name="junkd", bufs=2))
        junka = ctx.enter_context(tc.tile_pool(name="junka", bufs=2))
        prep_ctx = contextlib.ExitStack()
        tpsum = prep_ctx.enter_context(
            tc.tile_pool(name="tpsum", bufs=2, space="PSUM"))

        pT = singles.tile([128, B], BF16)         # full positives, transposed
        aT = singles.tile([128, SLAB], BF16)      # normalized anchors, transposed
        a_all = singles.tile([128, SLAB], BF16)   # raw anchor tiles (row-major)
        ah_all = singles.tile([128, SLAB], BF16)  # normalized anchors (row-major)
        ps_all = singles.tile([128, SLAB], BF16)  # own positive slab (row-major)
        ssq_a = singles.tile([128, AT], F32)
        rsq_a = singles.tile([128, AT], F32)
        ssq_p = singles.tile([128, AT], F32)
        dhat = singles.tile([128, AT], F32)
        racc_d = singles.tile([128, PT], F32)
        racc_a = singles.tile([128, PT], F32)
        junk_s = singles.tile([128, 128], BF16)
        from concourse.masks import make_identity
        ident = singles.tile([128, 128], BF16)
        make_identity(nc, ident[:])

        # per-partition-major DRAM views so multi-tile DMAs enumerate in
        # the same order as the SBUF destination [p, t, d]
        a_pm = a_in.rearrange("(n p) d -> p n d", p=128)
        ps_pm = ps_in.rearrange("(n p) d -> p n d", p=128)
        a3 = a_all.rearrange("p (n d) -> p n d", d=128)
        ps3 = ps_all.rearrange("p (n d) -> p n d", d=128)

        # ---- input DMAs.  XBAR transpose issues appear to block the
        # issuing sequencer until the transfer drains, so SP issues the
        # anchor loads first and then ONLY pT transposes; the positive
        # slab loads go out on the Act sequencer (idle during prep). ----
        for k in range(4):
            nc.sync.dma_start(out=a3[:, 2 * k:2 * k + 2, :],
                              in_=a_pm[:, 2 * k:2 * k + 2, :])
        for k in range(2):
            nc.scalar.dma_start(out=ps3[:, 4 * k:4 * k + 4, :],
                                in_=ps_pm[:, 4 * k:4 * k + 4, :])
        xw = B // XCH  # positive rows per xbar chunk
        for k in range(XCH):
            nc.sync.dma_start_transpose(
                pT[:, k * xw:(k + 1) * xw], p_in[k * xw:(k + 1) * xw, :])

        # ---- anchor normalization ----
        for t in range(AT):
            at = a_all[:, t * 128:(t + 1) * 128]
            nc.vector.scalar_tensor_tensor(
                out=junk_s[:], in0=at, scalar=1.0, in1=at,
                op0=byp, op1=mult, accum_out=ssq_a[:, t:t + 1])
        nc.scalar.activation(out=rsq_a[:], in_=ssq_a[:], func=Sqrt)
        nc.vector.reciprocal(out=rsq_a[:], in_=rsq_a[:])
        for t in range(AT):
            nc.vector.tensor_scalar(
                out=ah_all[:, t * 128:(t + 1) * 128],
                in0=a_all[:, t * 128:(t + 1) * 128],
                scalar1=rsq_a[:, t:t + 1], scalar2=None, op0=mult)
        # PE transposes for the anchor tiles (PE is idle during prep);
        # psum->SBUF copies on Act, which is also idle pre-main-loop
        for t in range(AT):
            tp = tpsum.tile([128, 128], BF16, tag="tp")
            nc.tensor.transpose(tp[:], ah_all[:, t * 128:(t + 1) * 128],
                                ident[:])
            nc.scalar.copy(out=aT[:, t * 128:(t + 1) * 128], in_=tp[:])
        prep_ctx.close()
        psum = ctx.enter_context(tc.tile_pool(name="psum", bufs=4, space="PSUM"))

        def _prep_stt(t):
            pst = ps_all[:, t * 128:(t + 1) * 128]
            nc.vector.scalar_tensor_tensor(
                out=junk_s[:], in0=pst, scalar=1.0, in1=pst,
                op0=byp, op1=mult, accum_out=ssq_p[:, t:t + 1])
            nc.vector.scalar_tensor_tensor(
                out=junk_s[:], in0=ah_all[:, t * 128:(t + 1) * 128],
                scalar=1.0, in1=pst,
                op0=byp, op1=mult, accum_out=dhat[:, t:t + 1])

        # ---- main loop: 64 blocks of [128 positives, 1024 anchors];
        # own-slab sumsq/diag stt ops are woven into the DVE stream after
        # its first couple of EW blocks ----
        stt_after = {6: [0, 1], 8: [2, 3], 10: [4, 5], 12: [6, 7]}
        for t in range(PT):
            ps = psum.tile([128, SLAB], F32, tag="mm")
            lhsT = pT[:, t * 128:(t + 1) * 128]
            for j in range(SLAB // MMN):
                nc.tensor.matmul(
                    out=ps[:, j * MMN:(j + 1) * MMN],
                    lhsT=lhsT, rhs=aT[:, j * MMN:(j + 1) * MMN],
                    start=True, stop=True)
            if t in DVE_BLOCKS:
                junk = junkd.tile([128, SLAB], BF16, tag="jd")
                nc.vector.tensor_scalar(
                    out=junk[:], in0=ps[:], scalar1=0.0, scalar2=None,
                    op0=amax, op1=add, accum_out=racc_d[:, t:t + 1])
            else:
                junk = junka.tile([128, SLAB], BF16, tag="ja")
                nc.scalar.activation(
                    out=junk[:], in_=ps[:], func=Relu,
                    accum_out=racc_a[:, t:t + 1])
            for pt in stt_after.get(t, []):
                _prep_stt(pt)

        # ---- outputs ----
        nc.sync.dma_start(out=racc_d_o[:], in_=racc_d[:])
        nc.sync.dma_start(out=racc_a_o[:], in_=racc_a[:])
        nc.sync.dma_start(out=ssq_p_o[:], in_=ssq_p[:])
        nc.sync.dma_start(out=dhat_o[:], in_=dhat[:])


def _build():
    nc = bacc.Bacc("TRN2", target_bir_lowering=False, debug=False,
                   num_devices=NCORES)
    a_in = nc.declare_dram_parameter("a", [SLAB, D], BF16, isOutput=False)
    p_in = nc.declare_dram_parameter("p", [B, D], BF16, isOutput=False)
    ps_in = nc.declare_dram_parameter("ps", [SLAB, D], BF16, isOutput=False)
    racc_d_o = nc.declare_dram_parameter("racc_d", [128, PT], F32, isOutput=True)
    racc_a_o = nc.declare_dram_parameter("racc_a", [128, PT], F32, isOutput=True)
    ssq_p_o = nc.declare_dram_parameter("ssq_p", [128, AT], F32, isOutput=True)
    dhat_o = nc.declare_dram_parameter("dhat", [128, AT], F32, isOutput=True)
    with tile.TileContext(nc) as tc:
        _body(tc, a_in[:], p_in[:], ps_in[:], racc_d_o[:], racc_a_o[:],
              ssq_p_o[:], dhat_o[:])
    nc.compile()
    return nc


def kernel(hid_positive: np.ndarray, hid_anchor: np.ndarray, **run_kwargs):
    if "nc" not in _CACHE:
        _CACHE["nc"] = _build()
    nc = _CACHE["nc"]
    p16 = np.asarray(hid_positive, dtype=np.float32).astype(ml_dtypes.bfloat16)
    a16 = np.asarray(hid_anchor, dtype=np.float32).astype(ml_dtypes.bfloat16)
    in_maps = []
    for c in range(NCORES):
        sl = slice(c * SLAB, (c + 1) * SLAB)
        in_maps.append({"a": a16[sl], "p": p16, "ps": p16[sl]})
    res = run_bass_kernel_spmd(nc, in_maps, core_ids=list(range(NCORES)),
                               **run_kwargs)
    # host: assemble 1/|p_j| from per-core slab sumsq
    ssq_full = np.empty(B, dtype=np.float64)
    for c in range(NCORES):
        arr = np.asarray(res.results[c]["ssq_p"], dtype=np.float64)  # [128, 8]
        ssq_full[c * SLAB:(c + 1) * SLAB] = arr.T.reshape(SLAB)
    rsq = 1.0 / np.maximum(np.sqrt(ssq_full), 1e-8)
    rsq_mat = rsq.reshape(PT, 128).T  # [128, 64]; [p, t] -> row 128t+p

    total = 0.0
    diag_relu = 0.0
    diag_cos = 0.0
    for c in range(NCORES):
        rd = np.asarray(res.results[c]["racc_d"], dtype=np.float64)
        ra = np.asarray(res.results[c]["racc_a"], dtype=np.float64)
        racc = ra
        for t in DVE_BLOCKS:
            racc[:, t] = rd[:, t]
        total += float((racc * rsq_mat).sum())
        dh = np.asarray(res.results[c]["dhat"], dtype=np.float64)  # [128, 8]
        # dhat[p, m] -> anchor/positive index 1024c + 128m + p
        r_slab = rsq[c * SLAB:(c + 1) * SLAB].reshape(AT, 128).T  # [128, 8]
        dcos = dh * r_slab
        diag_relu += float(np.maximum(dcos, 0.0).sum())
        diag_cos += float(dcos.sum())
    loss = (total - diag_relu - diag_cos + B) / (float(B) * float(B))
    if run_kwargs:
        _CACHE["last_result"] = res
    return np.asarray(loss, dtype=np.float32)
